# revision 22
# baseline (speedup 1.0000x reference)
"""Trainium2 Bass kernel for the dense GNN message-passing step.

Computation (N=16384, NUM_IN=1024, NUM_OUT=256):
    states = zeros(N); states[input_indices] = input_values
    total  = states @ W + biases                      # GEMV over [N, N] f32
    out    = act_select(total)[output_indices]        # 0=id, 1=relu, 2=softsign

Strategy:
  * `states` is zero outside the (<=1024) positions named by input_indices,
    so only those rows of W contribute to the GEMV. The host gathers the
    live rows and the device contracts over a padded K=1024 instead of
    16384 -> 16x less HBM traffic.
  * W is sharded column-wise across the 8 cores (tensor parallel): each
    core computes its 2048 outputs = GEMV slice + bias + per-neuron
    activation select; the host concatenates and gathers output_indices.
  * W is stored as single fp16 (2 B/element): the harness gate is
    rel_err < 2e-2 and the fp16 GEMV lands ~1e-4, so the fp32-exact hi/lo
    path (4 B/element) is 2x wasted HBM traffic. 4 MB/core total.
  * The 4 x 1MB W-chunk DMAs are issued back-to-back up front on the sync
    HWDGE queue: they drain FIFO on one ring, so chunk i completes ~3us
    after chunk i-1 at streaming rate and the PE starts ~4us in. Each
    chunk is [128, 8KB/partition] -> large descriptors at near line rate.
    (The old chained/pool-throttled scheme serialized each 512KB block
    with its ~2us completion latency: DMA idled 57%.)
  * x is stationary ([128,1] f16 per k-chunk), W moving ([128,512] f16),
    8 accumulating matmuls per chunk. Chunk nch's [1,512] strip lands at
    PSUM partition 32*nch of ONE bank (PE tile_position), so the epilogue
    runs on [2,512] stride-32 views: 2 batches x ~7 ops instead of
    4 x 10 single-partition ops. Bias is folded into the PSUM->SBUF move
    (DVE tensor_add), killing the ones-row bias matmuls.
  * Epilogue per batch: t = P + b; relu on ACT; a1 = |t|+1 (fused
    tensor_scalar abs_max+add); r = reciprocal_approx_fast(a1) (~18 bits,
    plenty for 2e-2); ss = t*r; two copy_predicated selects with
    host-precomputed uint8 masks. Batch {0,1} overlaps chunks 2-3.
"""

import numpy as np
from contextlib import ExitStack

import concourse.bacc as bacc
import concourse.tile as tile
from concourse import mybir
from concourse.bass_utils import run_bass_kernel_spmd

N_CORES = 8
K = 1024                 # padded contraction size (live rows)
KC = K // 128            # 8 k-chunks
NPC = 16384 // N_CORES   # 2048 output columns per core
NCH = NPC // 512         # 4 column chunks of 512
F32 = mybir.dt.float32
BF16 = mybir.dt.bfloat16
U8 = mybir.dt.uint8

_BUILT = None            # cached nc so repeat calls reuse the compiled module
LAST_RESULTS = None      # BassKernelResults of the most recent run (for test.py)


def _build_bass():
    nc = bacc.Bacc(
        "TRN2", target_bir_lowering=False, debug=False, num_devices=N_CORES
    )
    w = nc.dram_tensor("w", [NCH, 128, KC * 512], BF16, kind="ExternalInput").ap()
    # Stationary blocks: for (kc, j) a [128, 2] block whose column j is
    # x chunk kc and whose other column is zero. A matmul with this lhsT
    # writes a [2, 512] PSUM tile where row j accumulates x_kc' W and the
    # other row accumulates +0 — so chunk pair {2h, 2h+1} lands on
    # CONTIGUOUS partitions {0,1} of bank h (DVE cannot read strided
    # partitions, and the PE cannot place M=1 outputs at partition 1).
    xs = nc.dram_tensor("xs", [128, KC * 4], BF16, kind="ExternalInput").ap()
    # Epilogue operands packed [row(2), ...]: row j, col-block h holds chunk
    # 2h+j's values. bias f32; masks u8 (CopyPredicated requires int mask).
    aux = nc.dram_tensor("aux", [2, 1024], F32, kind="ExternalInput").ap()
    msk = nc.dram_tensor("msk", [2, 2 * 1024], U8, kind="ExternalInput").ap()
    o = nc.dram_tensor("o", [NCH, 512], F32, kind="ExternalOutput").ap()

    with tile.TileContext(nc) as tc:
        with ExitStack() as ctx:
            small = ctx.enter_context(tc.tile_pool(name="small", bufs=1))
            wpool = ctx.enter_context(tc.tile_pool(name="wp", bufs=NCH))
            ppool = ctx.enter_context(tc.tile_pool(name="pp", bufs=1, space="PSUM"))
            scr = ctx.enter_context(tc.tile_pool(name="scr", bufs=1))

            # xs first (16KB, gates the first matmul), then the 4 x 1MB W
            # chunks, all FIFO on the sync HWDGE queue.
            xs_t = small.tile([128, KC * 4], BF16, tag="xs")
            nc.sync.dma_start(xs_t[:], xs[:])
            wts = []
            for nch in range(NCH):
                wt = wpool.tile([128, KC * 512], BF16, tag="wblk")
                nc.sync.dma_start(wt[:], w[nch])
                wts.append(wt)

            # Epilogue operands in two DMAs on the scalar HWDGE queue
            # (needed ~10us later than xs).
            aux_t = small.tile([2, 1024], F32, tag="aux")
            nc.scalar.dma_start(aux_t[:], aux[:])
            msk_t = small.tile([2, 2 * 1024], U8, tag="msk")
            nc.scalar.dma_start(msk_t[:], msk[:])
            b_t = aux_t[:, 0:1024]
            m1_t = msk_t[:, 0:1024]
            m2_t = msk_t[:, 1024:2048]

            # PE warm-up: ~3.4us of dummy matmuls on a zeroed tile during
            # the otherwise-dead preamble window, so the HAM un-throttles
            # the PE (1.2 -> 2.4 GHz) before the real matmuls arrive.
            wu = scr.tile([128, 512], BF16, tag="wu")
            nc.gpsimd.memset(wu[:], 0.0)
            pw = ppool.tile([128, 512], F32, tag="pw")
            for _ in range(8):
                nc.tensor.matmul(
                    pw[0:1, :], wu[:, 0:1], wu[:], start=True, stop=True
                )

            # Chunk pair {2h, 2h+1} -> PSUM bank h rows {0,1}, one
            # 16-matmul accumulation group per bank.
            pt0 = ppool.tile([128, 512], F32, tag="p0")
            pt1 = ppool.tile([128, 512], F32, tag="p1")
            pts = [pt0, pt1]
            for half in range(2):
                pt = pts[half]
                for j in range(2):
                    nch = 2 * half + j
                    for kc in range(KC):
                        blk = (kc * 2 + j) * 2
                        nc.tensor.matmul(
                            pt[0:2, :],
                            xs_t[:, blk : blk + 2],
                            wts[nch][:, kc * 512 : (kc + 1) * 512],
                            start=(j == 0 and kc == 0),
                            stop=(j == 1 and kc == KC - 1),
                        )

            # Epilogue per pair on contiguous [2,512]; pair 0 overlaps
            # pair 1's matmuls.
            for half in range(2):
                cs = slice(half * 512, (half + 1) * 512)
                p2 = pts[half][0:2, :]
                ot = scr.tile([2, 512], F32, tag=f"ot{half}", name=f"ot{half}")
                rt = scr.tile([2, 512], F32, tag=f"rt{half}", name=f"rt{half}")
                at = scr.tile([2, 512], F32, tag=f"at{half}", name=f"at{half}")
                a1 = scr.tile([2, 512], F32, tag=f"a1{half}", name=f"a1{half}")
                rc = scr.tile([2, 512], F32, tag=f"rc{half}", name=f"rc{half}")
                ss = scr.tile([2, 512], F32, tag=f"ss{half}", name=f"ss{half}")
                nc.vector.tensor_add(ot[:], p2, b_t[:, cs])    # t = P + b
                nc.scalar.activation(                          # relu(t)
                    rt[:], ot[:], mybir.ActivationFunctionType.Relu
                )
                nc.scalar.activation(                          # |t|
                    at[:], ot[:], mybir.ActivationFunctionType.Abs
                )
                nc.scalar.activation(                          # 1 + |t|
                    a1[:], at[:], mybir.ActivationFunctionType.Copy, bias=1.0
                )
                nc.vector.reciprocal_approx_fast(rc[:], a1[:])
                nc.vector.tensor_mul(ss[:], ot[:], rc[:])      # softsign(t)
                nc.vector.copy_predicated(ot[:], m1_t[:, cs], rt[:])
                nc.vector.copy_predicated(ot[:], m2_t[:, cs], ss[:])
                nc.sync.dma_start(o[2 * half : 2 * half + 2], ot[:])

    nc.compile()
    return nc


def kernel(**inputs) -> np.ndarray:
    global _BUILT, LAST_RESULTS

    iv = np.asarray(inputs["input_values"], dtype=np.float32)
    W = np.asarray(inputs["weight_matrix"], dtype=np.float32)
    bias = np.asarray(inputs["biases"], dtype=np.float32)
    act = np.asarray(inputs["act_ids"])
    iidx = np.asarray(inputs["input_indices"]).astype(np.int64)
    oidx = np.asarray(inputs["output_indices"]).astype(np.int64)

    n = W.shape[0]
    # Dense neuron-state vector (duplicate indices: last write wins, matching
    # jax's .at[].set) and its index support.
    states = np.zeros(n, np.float32)
    states[iidx] = iv
    live = np.zeros(n, dtype=bool)
    live[iidx] = True
    support = np.flatnonzero(live)
    assert support.size <= K, "more than K live rows not supported"
    rows = np.zeros(K, np.int64)          # pad with row 0 (x=0 there => no-op)
    rows[: support.size] = support
    xvec = np.zeros(K, np.float32)
    xvec[: support.size] = states[support]

    import ml_dtypes

    bf16 = ml_dtypes.bfloat16
    Wh = W[rows].astype(bf16)             # [K, n] live rows, single bf16
    xh = xvec.astype(bf16)
    xc = xh.reshape(KC, 128).T            # [128, KC]
    # Stationary blocks [128, (kc*2+j)*2 + m]: x chunk kc in column m==j.
    xs_t = np.zeros((128, KC * 4), bf16)
    for kc in range(KC):
        for j in range(2):
            xs_t[:, (kc * 2 + j) * 2 + j] = xc[:, kc]

    in_maps = []
    for c in range(N_CORES):
        sl = slice(c * NPC, (c + 1) * NPC)
        wc = np.ascontiguousarray(
            Wh[:, sl].reshape(KC, 128, NCH, 512).transpose(2, 1, 0, 3)
        ).reshape(NCH, 128, KC * 512)
        def pack2(a):
            # [NCH*512] -> [row(2), half(2)*512]: packed[r, 512h+j] = chunk
            # (2h+r) col j, matching the b_t/m_t SBUF layout.
            return a.reshape(2, 2, 512).transpose(1, 0, 2).reshape(2, 1024)

        in_maps.append(
            {
                "w": wc,
                "xs": xs_t,
                "aux": np.ascontiguousarray(pack2(bias[sl].astype(np.float32))),
                "msk": np.ascontiguousarray(
                    np.concatenate(
                        [
                            pack2((act[sl] == 1).astype(np.uint8)),
                            pack2((act[sl] == 2).astype(np.uint8)),
                        ],
                        axis=1,
                    )
                ),
            }
        )

    if _BUILT is None:
        _BUILT = _build_bass()
    LAST_RESULTS = run_bass_kernel_spmd(
        _BUILT, in_maps, core_ids=list(range(N_CORES))
    )
    full = np.concatenate(
        [LAST_RESULTS.results[c]["o"].reshape(-1) for c in range(N_CORES)]
    )
    return full[oidx].astype(np.float32)


# revision 31
# speedup vs baseline: 1.0647x; 1.0647x over previous
"""Trainium2 Bass kernel for the dense GNN message-passing step.

Computation (N=16384, NUM_IN=1024, NUM_OUT=256):
    states = zeros(N); states[input_indices] = input_values
    total  = states @ W + biases                      # GEMV over [N, N] f32
    out    = act_select(total)[output_indices]        # 0=id, 1=relu, 2=softsign

Strategy:
  * `states` is zero outside the (<=1024) positions named by input_indices,
    so only those rows of W contribute to the GEMV. The host gathers the
    live rows and the device contracts over a padded K=1024 instead of
    16384 -> 16x less HBM traffic.
  * W is sharded column-wise across the 8 cores (tensor parallel): each
    core computes its 2048 outputs = GEMV slice + bias + per-neuron
    activation select; the host concatenates and gathers output_indices.
  * W is stored as single fp16 (2 B/element): the harness gate is
    rel_err < 2e-2 and the fp16 GEMV lands ~1e-4, so the fp32-exact hi/lo
    path (4 B/element) is 2x wasted HBM traffic. 4 MB/core total.
  * The 4 x 1MB W-chunk DMAs are issued back-to-back up front on the sync
    HWDGE queue: they drain FIFO on one ring, so chunk i completes ~3us
    after chunk i-1 at streaming rate and the PE starts ~4us in. Each
    chunk is [128, 8KB/partition] -> large descriptors at near line rate.
    (The old chained/pool-throttled scheme serialized each 512KB block
    with its ~2us completion latency: DMA idled 57%.)
  * x is stationary ([128,1] f16 per k-chunk), W moving ([128,512] f16),
    8 accumulating matmuls per chunk. Chunk nch's [1,512] strip lands at
    PSUM partition 32*nch of ONE bank (PE tile_position), so the epilogue
    runs on [2,512] stride-32 views: 2 batches x ~7 ops instead of
    4 x 10 single-partition ops. Bias is folded into the PSUM->SBUF move
    (DVE tensor_add), killing the ones-row bias matmuls.
  * Epilogue per batch: t = P + b; relu on ACT; a1 = |t|+1 (fused
    tensor_scalar abs_max+add); r = reciprocal_approx_fast(a1) (~18 bits,
    plenty for 2e-2); ss = t*r; two copy_predicated selects with
    host-precomputed uint8 masks. Batch {0,1} overlaps chunks 2-3.
"""

import numpy as np
from contextlib import ExitStack

import concourse.bacc as bacc
import concourse.tile as tile
from concourse import mybir
from concourse.bass_utils import run_bass_kernel_spmd

N_CORES = 8
K = 1024                 # padded contraction size (live rows)
KC = K // 128            # 8 k-chunks
NPC = 16384 // N_CORES   # 2048 output columns per core
NCH = NPC // 512         # 4 column chunks of 512
F32 = mybir.dt.float32
F16 = mybir.dt.float16
U8 = mybir.dt.uint8

_BUILT = None            # cached nc so repeat calls reuse the compiled module
LAST_RESULTS = None      # BassKernelResults of the most recent run (for test.py)


def _build_bass():
    nc = bacc.Bacc(
        "TRN2", target_bir_lowering=False, debug=False, num_devices=N_CORES
    )
    w = nc.dram_tensor("w", [NCH, 128, KC * 512], F16, kind="ExternalInput").ap()
    # Stationary blocks: for (kc, j) a [128, 2] block whose column j is
    # x chunk kc and whose other column is zero. A matmul with this lhsT
    # writes a [2, 512] PSUM tile where row j accumulates x_kc' W and the
    # other row accumulates +0 — so chunk pair {2h, 2h+1} lands on
    # CONTIGUOUS partitions {0,1} of bank h (DVE cannot read strided
    # partitions, and the PE cannot place M=1 outputs at partition 1).
    xs = nc.dram_tensor("xs", [128, KC * 4], F16, kind="ExternalInput").ap()
    # Epilogue operands packed [row(2), ...]: row j, col-block h holds chunk
    # 2h+j's values. bias f32; masks u8 (CopyPredicated requires int mask).
    aux = nc.dram_tensor("aux", [2, 1024], F32, kind="ExternalInput").ap()
    msk = nc.dram_tensor("msk", [2, 2 * 1024], U8, kind="ExternalInput").ap()
    o = nc.dram_tensor("o", [NCH, 512], F32, kind="ExternalOutput").ap()

    with tile.TileContext(nc) as tc:
        with ExitStack() as ctx:
            small = ctx.enter_context(tc.tile_pool(name="small", bufs=1))
            wpool = ctx.enter_context(tc.tile_pool(name="wp", bufs=NCH))
            ppool = ctx.enter_context(tc.tile_pool(name="pp", bufs=1, space="PSUM"))
            scr = ctx.enter_context(tc.tile_pool(name="scr", bufs=1))

            # xs first (16KB, gates the first matmul), then the 4 x 1MB W
            # chunks, all FIFO on the sync HWDGE queue.
            xs_t = small.tile([128, KC * 4], F16, tag="xs")
            nc.sync.dma_start(xs_t[:], xs[:])
            wts = []
            for nch in range(NCH):
                wt = wpool.tile([128, KC * 512], F16, tag="wblk")
                nc.sync.dma_start(wt[:], w[nch])
                wts.append(wt)

            # Epilogue operands in two DMAs on the scalar HWDGE queue
            # (needed ~10us later than xs).
            aux_t = small.tile([2, 1024], F32, tag="aux")
            nc.scalar.dma_start(aux_t[:], aux[:])
            msk_t = small.tile([2, 2 * 1024], U8, tag="msk")
            nc.scalar.dma_start(msk_t[:], msk[:])
            b_t = aux_t[:, 0:1024]
            m1_t = msk_t[:, 0:1024]
            m2_t = msk_t[:, 1024:2048]
            # PE warm-up: small dummy matmuls fill the dead preamble window
            # (~7.2-11us) with PE activity so the HAM un-throttles the clock
            # (1.2 -> 2.4 GHz) BEFORE W chunk 0 lands (~12us). Warm matmuls
            # pipeline at ~215ns (LDW hidden); cold ones cost 634ns and eat
            # the pair-0/pair-1 epilogue stagger.
            wu = scr.tile([128, 64], F16, tag="wu")
            nc.gpsimd.memset(wu[:], 0.0)
            pw = ppool.tile([128, 512], F32, tag="pw")
            for _ in range(20):
                nc.tensor.matmul(
                    pw[0:1, 0:64], wu[:, 0:1], wu[:], start=True, stop=True
                )

            # Chunk pair {2h, 2h+1} -> PSUM bank h rows {0,1}, one
            # 16-matmul accumulation group per bank.
            pt0 = ppool.tile([128, 512], F32, tag="p0")
            pt1 = ppool.tile([128, 512], F32, tag="p1")
            pts = [pt0, pt1]
            for half in range(2):
                pt = pts[half]
                for j in range(2):
                    nch = 2 * half + j
                    for kc in range(KC):
                        blk = (kc * 2 + j) * 2
                        nc.tensor.matmul(
                            pt[0:2, :],
                            xs_t[:, blk : blk + 2],
                            wts[nch][:, kc * 512 : (kc + 1) * 512],
                            start=(j == 0 and kc == 0),
                            stop=(j == 1 and kc == KC - 1),
                        )

            # Epilogue per pair on contiguous [2,512]; pair 0 overlaps
            # pair 1's matmuls.
            for half in range(2):
                cs = slice(half * 512, (half + 1) * 512)
                p2 = pts[half][0:2, :]
                ot = scr.tile([2, 512], F32, tag=f"ot{half}", name=f"ot{half}")
                rt = scr.tile([2, 512], F32, tag=f"rt{half}", name=f"rt{half}")
                a1 = scr.tile([2, 512], F32, tag=f"a1{half}", name=f"a1{half}")
                rc = scr.tile([2, 512], F32, tag=f"rc{half}", name=f"rc{half}")
                ss = scr.tile([2, 512], F32, tag=f"ss{half}", name=f"ss{half}")
                at = scr.tile([2, 512], F32, tag=f"at{half}", name=f"at{half}")
                nc.vector.tensor_add(ot[:], p2, b_t[:, cs])    # t = P + b
                nc.scalar.activation(                          # relu(t)
                    rt[:], ot[:], mybir.ActivationFunctionType.Relu
                )
                nc.scalar.activation(                          # |t|
                    at[:], ot[:], mybir.ActivationFunctionType.Abs
                )
                nc.scalar.activation(                          # 1 + |t|
                    a1[:], at[:], mybir.ActivationFunctionType.Copy, bias=1.0
                )
                nc.vector.reciprocal_approx_fast(rc[:], a1[:])
                nc.vector.tensor_mul(ss[:], ot[:], rc[:])      # softsign(t)
                nc.vector.copy_predicated(ot[:], m1_t[:, cs], rt[:])
                nc.vector.copy_predicated(ot[:], m2_t[:, cs], ss[:])
                nc.sync.dma_start(o[2 * half : 2 * half + 2], ot[:])

    nc.compile()
    return nc


def kernel(**inputs) -> np.ndarray:
    global _BUILT, LAST_RESULTS

    iv = np.asarray(inputs["input_values"], dtype=np.float32)
    W = np.asarray(inputs["weight_matrix"], dtype=np.float32)
    bias = np.asarray(inputs["biases"], dtype=np.float32)
    act = np.asarray(inputs["act_ids"])
    iidx = np.asarray(inputs["input_indices"]).astype(np.int64)
    oidx = np.asarray(inputs["output_indices"]).astype(np.int64)

    n = W.shape[0]
    # Dense neuron-state vector (duplicate indices: last write wins, matching
    # jax's .at[].set) and its index support.
    states = np.zeros(n, np.float32)
    states[iidx] = iv
    live = np.zeros(n, dtype=bool)
    live[iidx] = True
    support = np.flatnonzero(live)
    assert support.size <= K, "more than K live rows not supported"
    rows = np.zeros(K, np.int64)          # pad with row 0 (x=0 there => no-op)
    rows[: support.size] = support
    xvec = np.zeros(K, np.float32)
    xvec[: support.size] = states[support]

    Wh = W[rows].astype(np.float16)       # [K, n] live rows, single fp16
    xh = xvec.astype(np.float16)
    xc = xh.reshape(KC, 128).T            # [128, KC]
    # Stationary blocks [128, (kc*2+j)*2 + m]: x chunk kc in column m==j.
    xs_t = np.zeros((128, KC * 4), np.float16)
    for kc in range(KC):
        for j in range(2):
            xs_t[:, (kc * 2 + j) * 2 + j] = xc[:, kc]

    in_maps = []
    for c in range(N_CORES):
        sl = slice(c * NPC, (c + 1) * NPC)
        wc = np.ascontiguousarray(
            Wh[:, sl].reshape(KC, 128, NCH, 512).transpose(2, 1, 0, 3)
        ).reshape(NCH, 128, KC * 512)
        def pack2(a):
            # [NCH*512] -> [row(2), half(2)*512]: packed[r, 512h+j] = chunk
            # (2h+r) col j, matching the b_t/m_t SBUF layout.
            return a.reshape(2, 2, 512).transpose(1, 0, 2).reshape(2, 1024)

        in_maps.append(
            {
                "w": wc,
                "xs": xs_t,
                "aux": np.ascontiguousarray(pack2(bias[sl].astype(np.float32))),
                "msk": np.ascontiguousarray(
                    np.concatenate(
                        [
                            pack2((act[sl] == 1).astype(np.uint8)),
                            pack2((act[sl] == 2).astype(np.uint8)),
                        ],
                        axis=1,
                    )
                ),
            }
        )

    if _BUILT is None:
        _BUILT = _build_bass()
    LAST_RESULTS = run_bass_kernel_spmd(
        _BUILT, in_maps, core_ids=list(range(N_CORES))
    )
    full = np.concatenate(
        [LAST_RESULTS.results[c]["o"].reshape(-1) for c in range(N_CORES)]
    )
    return full[oidx].astype(np.float32)


# revision 32
# speedup vs baseline: 1.2324x; 1.1576x over previous
"""Trainium2 Bass kernel for the dense GNN message-passing step.

Computation (N=16384, NUM_IN=1024, NUM_OUT=256):
    states = zeros(N); states[input_indices] = input_values
    total  = states @ W + biases                      # GEMV over [N, N] f32
    out    = act_select(total)[output_indices]        # 0=id, 1=relu, 2=softsign

Strategy:
  * `states` is zero outside the (<=1024) positions named by input_indices,
    so only those rows of W contribute to the GEMV. The host gathers the
    live rows and the device contracts over a padded K=1024 instead of
    16384 -> 16x less HBM traffic.
  * W is sharded column-wise across the 8 cores (tensor parallel): each
    core computes its 2048 outputs = GEMV slice + bias + per-neuron
    activation select; the host concatenates and gathers output_indices.
  * W is stored as fp8 e3m4 scaled by 64 (1 B/element; rel err ~7e-3 vs
    the 2e-2 gate); x is folded by 1/64 in fp16 (exact power-of-2), so
    x' W is computed with no device-side descale. 2 MB/core total HBM.
  * The 2 x 1MB W-chunk DMAs are issued back-to-back up front on the sync
    HWDGE queue (FIFO, 8KB/partition descriptors ~ line rate); chunk h
    covers chunk-pair h so pair 0's matmuls start ~1 chunk in.
  * x is stationary: for (kc, j) a [128, 2] fp16 block with x chunk kc in
    column j, zero in the other. Each matmul accumulates a [2, 512] PSUM
    tile (row j real, +0 elsewhere), so pair {2h, 2h+1} lands on
    contiguous partitions {0,1} of bank h (DVE cannot read strided
    partitions; PE output base partition is restricted to {0,32,64}).
  * ~3.4us of dummy warm-up matmuls fill the dead preamble window so the
    HAM un-throttles the PE clock (1.2 -> 2.4 GHz) before W arrives; warm
    matmuls pipeline at ~216ns vs 634ns cold.
  * Select-free epilogue (7 ops/pair, no copy_predicated, no masks):
        t  = P + b
        tm = t * m2f          (m2f = 1 where softsign else 0)
        rc = 1/(1 + |tm|)     (ACT Abs + ACT Copy(bias=1) + DVE recip)
        y  = max(t * rc, L)   (L = 0 where relu else -3e38)
    For softsign neurons rc = 1/(1+|t|) and L=-inf -> t/(1+|t|); for relu
    neurons rc = 1 and L=0 -> max(t,0); else rc=1, L=-inf -> t.
"""

import numpy as np
from contextlib import ExitStack

import concourse.bacc as bacc
import concourse.tile as tile
from concourse import mybir
from concourse.bass_utils import run_bass_kernel_spmd

N_CORES = 8
K = 1024                 # padded contraction size (live rows)
KC = K // 128            # 8 k-chunks
NPC = 16384 // N_CORES   # 2048 output columns per core
NCH = NPC // 512         # 4 column chunks of 512
WS = 64.0                # fp8 weight scale (power of 2; x carries 1/WS)
F32 = mybir.dt.float32
F16 = mybir.dt.float16
F8 = mybir.dt.float8e3
U8 = mybir.dt.uint8

_BUILT = None            # cached nc so repeat calls reuse the compiled module
LAST_RESULTS = None      # BassKernelResults of the most recent run (for test.py)


def _build_bass():
    nc = bacc.Bacc(
        "TRN2", target_bir_lowering=False, debug=False, num_devices=N_CORES
    )
    # w[h]: pair h's weights, [p, (j*KC + kc)*512 + c] = W[kc*128+p,
    # 1024h + 512j + c] (x64, e3m4).
    w = nc.dram_tensor(
        "w", [2, 128, 2 * KC * 512], F8, kind="ExternalInput"
    ).ap()
    xs = nc.dram_tensor("xs", [128, KC * 4], F16, kind="ExternalInput").ap()
    # aux packed [row(2), b|L|m2f x half(2)*512] f32: row j, col-block h
    # holds chunk 2h+j's values.
    aux = nc.dram_tensor("aux", [2, 3 * 1024], F32, kind="ExternalInput").ap()
    o = nc.dram_tensor("o", [NCH, 512], F32, kind="ExternalOutput").ap()

    with tile.TileContext(nc) as tc:
        with ExitStack() as ctx:
            small = ctx.enter_context(tc.tile_pool(name="small", bufs=1))
            wpool = ctx.enter_context(tc.tile_pool(name="wp", bufs=2))
            ppool = ctx.enter_context(tc.tile_pool(name="pp", bufs=1, space="PSUM"))
            scr = ctx.enter_context(tc.tile_pool(name="scr", bufs=1))

            # xs first (16KB, gates the first matmul), then the 2 x 1MB W
            # chunks, all FIFO on the sync HWDGE queue.
            xs_t = small.tile([128, KC * 4], F16, tag="xs")
            nc.sync.dma_start(xs_t[:], xs[:])
            wts = []
            for h in range(2):
                wt = wpool.tile([128, 2 * KC * 512], F8, tag="wblk")
                nc.sync.dma_start(wt[:], w[h])
                wts.append(wt)

            # Epilogue operands in one DMA on the scalar HWDGE queue.
            aux_t = small.tile([2, 3 * 1024], F32, tag="aux")
            nc.scalar.dma_start(aux_t[:], aux[:])
            b_t = aux_t[:, 0:1024]
            l_t = aux_t[:, 1024:2048]
            m2_t = aux_t[:, 2048:3072]

            # PE warm-up: ~3.4us of dummy matmuls during the dead preamble
            # window so the HAM un-throttles the PE clock before W arrives.
            wu = scr.tile([128, 512], F16, tag="wu")
            nc.gpsimd.memset(wu[:], 0.0)
            pw = ppool.tile([128, 512], F32, tag="pw")
            for _ in range(8):
                nc.tensor.matmul(
                    pw[0:1, :], wu[:, 0:1], wu[:], start=True, stop=True
                )

            # Chunk pair {2h, 2h+1} -> PSUM bank h rows {0,1}, one
            # 16-matmul accumulation group per bank.
            pt0 = ppool.tile([128, 512], F32, tag="p0")
            pt1 = ppool.tile([128, 512], F32, tag="p1")
            pts = [pt0, pt1]
            for half in range(2):
                pt = pts[half]
                for j in range(2):
                    for kc in range(KC):
                        blk = (kc * 2 + j) * 2
                        nc.tensor.matmul(
                            pt[0:2, :],
                            xs_t[:, blk : blk + 2],
                            wts[half][:, (j * KC + kc) * 512 : (j * KC + kc + 1) * 512],
                            start=(j == 0 and kc == 0),
                            stop=(j == 1 and kc == KC - 1),
                        )

            # Epilogue per pair on contiguous [2,512]; pair 0 overlaps
            # pair 1's matmuls.
            for half in range(2):
                cs = slice(half * 512, (half + 1) * 512)
                p2 = pts[half][0:2, :]
                ot = scr.tile([2, 512], F32, tag=f"ot{half}", name=f"ot{half}")
                tm = scr.tile([2, 512], F32, tag=f"tm{half}", name=f"tm{half}")
                at = scr.tile([2, 512], F32, tag=f"at{half}", name=f"at{half}")
                a1 = scr.tile([2, 512], F32, tag=f"a1{half}", name=f"a1{half}")
                rc = scr.tile([2, 512], F32, tag=f"rc{half}", name=f"rc{half}")
                sf = scr.tile([2, 512], F32, tag=f"sf{half}", name=f"sf{half}")
                nc.vector.tensor_add(ot[:], p2, b_t[:, cs])    # t = P + b
                nc.vector.tensor_mul(tm[:], ot[:], m2_t[:, cs])
                nc.scalar.activation(                          # |t*m2f|
                    at[:], tm[:], mybir.ActivationFunctionType.Abs
                )
                nc.scalar.activation(                          # 1 + |t*m2f|
                    a1[:], at[:], mybir.ActivationFunctionType.Copy, bias=1.0
                )
                nc.vector.reciprocal_approx_fast(rc[:], a1[:])
                nc.vector.tensor_mul(sf[:], ot[:], rc[:])      # t * rc
                nc.vector.tensor_max(sf[:], sf[:], l_t[:, cs])  # relu select
                nc.sync.dma_start(o[2 * half : 2 * half + 2], sf[:])

    nc.compile()
    return nc


def kernel(**inputs) -> np.ndarray:
    global _BUILT, LAST_RESULTS

    import ml_dtypes

    iv = np.asarray(inputs["input_values"], dtype=np.float32)
    W = np.asarray(inputs["weight_matrix"], dtype=np.float32)
    bias = np.asarray(inputs["biases"], dtype=np.float32)
    act = np.asarray(inputs["act_ids"])
    iidx = np.asarray(inputs["input_indices"]).astype(np.int64)
    oidx = np.asarray(inputs["output_indices"]).astype(np.int64)

    n = W.shape[0]
    # Dense neuron-state vector (duplicate indices: last write wins, matching
    # jax's .at[].set) and its index support.
    states = np.zeros(n, np.float32)
    states[iidx] = iv
    live = np.zeros(n, dtype=bool)
    live[iidx] = True
    support = np.flatnonzero(live)
    assert support.size <= K, "more than K live rows not supported"
    rows = np.zeros(K, np.int64)          # pad with row 0 (x=0 there => no-op)
    rows[: support.size] = support
    xvec = np.zeros(K, np.float32)
    xvec[: support.size] = states[support]

    Wq = (W[rows] * WS).astype(ml_dtypes.float8_e3m4)   # [K, n] live rows
    xh = (xvec / WS).astype(np.float16)
    xc = xh.reshape(KC, 128).T            # [128, KC]
    # Stationary blocks [128, (kc*2+j)*2 + m]: x chunk kc in column m==j.
    xs_t = np.zeros((128, KC * 4), np.float16)
    for kc in range(KC):
        for j in range(2):
            xs_t[:, (kc * 2 + j) * 2 + j] = xc[:, kc]

    lsel = np.where(act == 1, 0.0, -3.0e38).astype(np.float32)
    m2f = (act == 2).astype(np.float32)

    in_maps = []
    for c in range(N_CORES):
        sl = slice(c * NPC, (c + 1) * NPC)
        # [kc, p, h, j, c] -> [h, p, j, kc, c]
        wc = np.ascontiguousarray(
            Wq[:, sl].reshape(KC, 128, 2, 2, 512).transpose(2, 1, 3, 0, 4)
        ).reshape(2, 128, 2 * KC * 512)

        def pack2(a):
            # [NCH*512] -> [row(2), half(2)*512]: packed[r, 512h+j] = chunk
            # (2h+r) col j, matching the aux SBUF layout.
            return a.reshape(2, 2, 512).transpose(1, 0, 2).reshape(2, 1024)

        in_maps.append(
            {
                "w": wc,
                "xs": xs_t,
                "aux": np.ascontiguousarray(
                    np.concatenate(
                        [
                            pack2(bias[sl].astype(np.float32)),
                            pack2(lsel[sl]),
                            pack2(m2f[sl]),
                        ],
                        axis=1,
                    )
                ),
            }
        )

    if _BUILT is None:
        _BUILT = _build_bass()
    LAST_RESULTS = run_bass_kernel_spmd(
        _BUILT, in_maps, core_ids=list(range(N_CORES))
    )
    full = np.concatenate(
        [LAST_RESULTS.results[c]["o"].reshape(-1) for c in range(N_CORES)]
    )
    return full[oidx].astype(np.float32)


# revision 39
# speedup vs baseline: 1.2781x; 1.0371x over previous
"""Trainium2 Bass kernel for the dense GNN message-passing step.

Computation (N=16384, NUM_IN=1024, NUM_OUT=256):
    states = zeros(N); states[input_indices] = input_values
    total  = states @ W + biases                      # GEMV over [N, N] f32
    out    = act_select(total)[output_indices]        # 0=id, 1=relu, 2=softsign

Strategy:
  * `states` is zero outside the (<=1024) positions named by input_indices,
    so only those rows of W contribute to the GEMV. The host gathers the
    live rows and the device contracts over a padded K=1024 instead of
    16384 -> 16x less HBM traffic.
  * W is sharded column-wise across the 8 cores (tensor parallel): each
    core computes its 2048 outputs = GEMV slice + bias + per-neuron
    activation select; the host concatenates and gathers output_indices.
  * W is stored as fp8 e3m4 scaled by 64 (1 B/element; rel err ~7e-3 vs
    the 2e-2 gate); x is folded by 1/64 in fp16 (exact power-of-2), so
    x' W is computed with no device-side descale. 2 MB/core total HBM.
  * The 2 x 1MB W-chunk DMAs are issued back-to-back up front on the sync
    HWDGE queue (FIFO, 8KB/partition descriptors ~ line rate); chunk h
    covers chunk-pair h so pair 0's matmuls start ~1 chunk in.
  * x is stationary: for (kc, j) a [128, 2] fp16 block with x chunk kc in
    column j, zero in the other. Each matmul accumulates a [2, 512] PSUM
    tile (row j real, +0 elsewhere), so pair {2h, 2h+1} lands on
    contiguous partitions {0,1} of bank h (DVE cannot read strided
    partitions; PE output base partition is restricted to {0,32,64}).
  * ~3.4us of dummy warm-up matmuls fill the dead preamble window so the
    HAM un-throttles the PE clock (1.2 -> 2.4 GHz) before W arrives; warm
    matmuls pipeline at ~216ns vs 634ns cold.
  * Select-free epilogue (7 ops/pair, no copy_predicated, no masks):
        t  = P + b
        tm = t * m2f          (m2f = 1 where softsign else 0)
        rc = 1/(1 + |tm|)     (ACT Abs + ACT Copy(bias=1) + DVE recip)
        y  = max(t * rc, L)   (L = 0 where relu else -3e38)
    For softsign neurons rc = 1/(1+|t|) and L=-inf -> t/(1+|t|); for relu
    neurons rc = 1 and L=0 -> max(t,0); else rc=1, L=-inf -> t.
"""

import numpy as np
from contextlib import ExitStack

import concourse.bacc as bacc
import concourse.tile as tile
from concourse import mybir
from concourse.bass_utils import run_bass_kernel_spmd

N_CORES = 8
K = 1024                 # padded contraction size (live rows)
KC = K // 128            # 8 k-chunks
NPC = 16384 // N_CORES   # 2048 output columns per core
NCH = NPC // 512         # 4 column chunks of 512
WS = 64.0                # fp8 weight scale (power of 2; x carries 1/WS)
F32 = mybir.dt.float32
F16 = mybir.dt.float16
F8 = mybir.dt.float8e3
U8 = mybir.dt.uint8

_BUILT = None            # cached nc so repeat calls reuse the compiled module
LAST_RESULTS = None      # BassKernelResults of the most recent run (for test.py)


def _build_bass():
    nc = bacc.Bacc(
        "TRN2", target_bir_lowering=False, debug=False, num_devices=N_CORES
    )
    # Pair 0 ships as two 512KB quarter-chunks (j=0, j=1) so its matmuls
    # (and epilogue) start earlier; pair 1 as one 1MB chunk.
    # w0[j]: [p, kc*512 + c] = W[kc*128+p, 512j + c] (x64, e3m4).
    # w1: [p, (j*KC+kc)*512 + c] = W[kc*128+p, 1024 + 512j + c].
    w0 = nc.dram_tensor("w0", [2, 128, KC * 512], F8, kind="ExternalInput").ap()
    w1 = nc.dram_tensor("w1", [128, 2 * KC * 512], F8, kind="ExternalInput").ap()
    # xs cols 0..63: stationary x blocks; cols 64..65: a [2,2] identity
    # for the K=2 bias matmul.
    xs = nc.dram_tensor("xs", [128, KC * 4 + 2], F16, kind="ExternalInput").ap()
    # bias rows packed [row(2), half(2)*512] f16 — moving operand of the
    # bias matmul.
    bh = nc.dram_tensor("bh", [2, 2 * 512], F16, kind="ExternalInput").ap()
    # aux packed [row(2), L|m2f x half(2)*512] f32: row j, col-block h
    # holds chunk 2h+j's values.
    aux = nc.dram_tensor("aux", [2, 2 * 1024], F32, kind="ExternalInput").ap()
    o = nc.dram_tensor("o", [NCH, 512], F32, kind="ExternalOutput").ap()

    with tile.TileContext(nc) as tc:
        with ExitStack() as ctx:
            small = ctx.enter_context(tc.tile_pool(name="small", bufs=1))
            wpool = ctx.enter_context(tc.tile_pool(name="wp", bufs=2))
            ppool = ctx.enter_context(tc.tile_pool(name="pp", bufs=1, space="PSUM"))
            scr = ctx.enter_context(tc.tile_pool(name="scr", bufs=1))

            # W chunks FIFO on the sync HWDGE queue: pair 0 as two 512KB
            # quarters, pair 1 as one 1MB chunk.
            wts = []
            for q in range(2):
                wt = wpool.tile([128, KC * 512], F8, tag="wq", name=f"wq{q}")
                nc.sync.dma_start(wt[:], w0[q])
                wts.append(wt)
            w1_t = wpool.tile([128, 2 * KC * 512], F8, tag="w1")
            nc.sync.dma_start(w1_t[:], w1[:])

            def wslice(half, j, kc):
                if half == 0:
                    return wts[j][:, kc * 512 : (kc + 1) * 512]
                return w1_t[:, (j * KC + kc) * 512 : (j * KC + kc + 1) * 512]

            # Small tensors on the scalar HWDGE queue, xs first (it gates
            # the first matmul).
            xs_t = small.tile([128, KC * 4 + 2], F16, tag="xs")
            nc.scalar.dma_start(xs_t[:], xs[:])
            bh_t = small.tile([2, 2 * 512], F16, tag="bh")
            nc.scalar.dma_start(bh_t[:], bh[:])
            aux_t = small.tile([2, 2 * 1024], F32, tag="aux")
            nc.scalar.dma_start(aux_t[:], aux[:])
            l_t = aux_t[:, 0:1024]
            m2_t = aux_t[:, 1024:2048]

            # GpSimd elementwise cost probe (dead code, off every chain).
            gp = scr.tile([2, 512], F32, tag="gp")
            nc.gpsimd.tensor_add(gp[:], aux_t[:, 0:512], aux_t[:, 512:1024])

            # PE warm-up: ~3.4us of dummy matmuls during the dead preamble
            # window so the HAM un-throttles the PE clock before W arrives.
            wu = scr.tile([128, 512], F16, tag="wu")
            nc.gpsimd.memset(wu[:], 0.0)
            pw = ppool.tile([128, 512], F32, tag="pw")
            for _ in range(8):
                nc.tensor.matmul(
                    pw[0:1, :], wu[:, 0:1], wu[:], start=True, stop=True
                )

            # Chunk pair {2h, 2h+1} -> PSUM bank h rows {0,1}: one
            # accumulation group per bank = K=2 bias matmul (identity
            # stationary x [2,512] bias rows, start=True) + 16 x-matmuls.
            # After the group closes, PSUM holds t = x'W + b directly.
            eye2 = xs_t[0:2, KC * 4 : KC * 4 + 2]
            pt0 = ppool.tile([128, 512], F32, tag="p0")
            pt1 = ppool.tile([128, 512], F32, tag="p1")
            pts = [pt0, pt1]
            for half in range(2):
                cs = slice(half * 512, (half + 1) * 512)
                pt = pts[half]
                nc.tensor.matmul(
                    pt[0:2, :], eye2, bh_t[:, cs], start=True, stop=False
                )
                for j in range(2):
                    for kc in range(KC):
                        blk = (kc * 2 + j) * 2
                        nc.tensor.matmul(
                            pt[0:2, :],
                            xs_t[:, blk : blk + 2],
                            wslice(half, j, kc),
                            start=False,
                            stop=(j == 1 and kc == KC - 1),
                        )

            # Epilogue per pair on contiguous [2,512], reading t from PSUM;
            # pair 0 overlaps pair 1's matmuls.
            for half in range(2):
                cs = slice(half * 512, (half + 1) * 512)
                p2 = pts[half][0:2, :]
                tm = scr.tile([2, 512], F32, tag=f"tm{half}", name=f"tm{half}")
                at = scr.tile([2, 512], F32, tag=f"at{half}", name=f"at{half}")
                a1 = scr.tile([2, 512], F32, tag=f"a1{half}", name=f"a1{half}")
                rc = scr.tile([2, 512], F32, tag=f"rc{half}", name=f"rc{half}")
                sf = scr.tile([2, 512], F32, tag=f"sf{half}", name=f"sf{half}")
                nc.vector.tensor_mul(tm[:], p2, m2_t[:, cs])   # t * m2f
                nc.scalar.activation(                          # |t*m2f|
                    at[:], tm[:], mybir.ActivationFunctionType.Abs
                )
                nc.scalar.activation(                          # 1 + |t*m2f|
                    a1[:], at[:], mybir.ActivationFunctionType.Copy, bias=1.0
                )
                nc.vector.reciprocal_approx_fast(rc[:], a1[:])
                nc.vector.tensor_mul(sf[:], p2, rc[:])         # t * rc
                nc.vector.tensor_max(sf[:], sf[:], l_t[:, cs])  # relu select
                nc.sync.dma_start(o[2 * half : 2 * half + 2], sf[:])

    nc.compile()
    return nc


def kernel(**inputs) -> np.ndarray:
    global _BUILT, LAST_RESULTS

    import ml_dtypes

    iv = np.asarray(inputs["input_values"], dtype=np.float32)
    W = np.asarray(inputs["weight_matrix"], dtype=np.float32)
    bias = np.asarray(inputs["biases"], dtype=np.float32)
    act = np.asarray(inputs["act_ids"])
    iidx = np.asarray(inputs["input_indices"]).astype(np.int64)
    oidx = np.asarray(inputs["output_indices"]).astype(np.int64)

    n = W.shape[0]
    # Dense neuron-state vector (duplicate indices: last write wins, matching
    # jax's .at[].set) and its index support.
    states = np.zeros(n, np.float32)
    states[iidx] = iv
    live = np.zeros(n, dtype=bool)
    live[iidx] = True
    support = np.flatnonzero(live)
    assert support.size <= K, "more than K live rows not supported"
    rows = np.zeros(K, np.int64)          # pad with row 0 (x=0 there => no-op)
    rows[: support.size] = support
    xvec = np.zeros(K, np.float32)
    xvec[: support.size] = states[support]

    Wq = (W[rows] * WS).astype(ml_dtypes.float8_e3m4)   # [K, n] live rows
    xh = (xvec / WS).astype(np.float16)
    xc = xh.reshape(KC, 128).T            # [128, KC]
    # Stationary blocks [128, (kc*2+j)*2 + m]: x chunk kc in column m==j;
    # trailing [2,2] identity for the bias matmul.
    xs_t = np.zeros((128, KC * 4 + 2), np.float16)
    for kc in range(KC):
        for j in range(2):
            xs_t[:, (kc * 2 + j) * 2 + j] = xc[:, kc]
    xs_t[0, KC * 4] = 1.0
    xs_t[1, KC * 4 + 1] = 1.0

    lsel = np.where(act == 1, 0.0, -3.0e38).astype(np.float32)
    m2f = (act == 2).astype(np.float32)

    in_maps = []
    for c in range(N_CORES):
        sl = slice(c * NPC, (c + 1) * NPC)
        # [kc, p, h, j, c] -> [h, p, j, kc, c]
        wc = Wq[:, sl].reshape(KC, 128, 2, 2, 512).transpose(2, 1, 3, 0, 4)

        def pack2(a):
            # [NCH*512] -> [row(2), half(2)*512]: packed[r, 512h+j] = chunk
            # (2h+r) col j, matching the aux SBUF layout.
            return a.reshape(2, 2, 512).transpose(1, 0, 2).reshape(2, 1024)

        in_maps.append(
            {
                "w0": np.ascontiguousarray(
                    wc[0].transpose(1, 0, 2, 3)     # [j, p, kc, c]
                ).reshape(2, 128, KC * 512),
                "w1": np.ascontiguousarray(wc[1]).reshape(128, 2 * KC * 512),
                "xs": xs_t,
                "bh": np.ascontiguousarray(
                    pack2(bias[sl].astype(np.float16))
                ),
                "aux": np.ascontiguousarray(
                    np.concatenate([pack2(lsel[sl]), pack2(m2f[sl])], axis=1)
                ),
            }
        )

    if _BUILT is None:
        _BUILT = _build_bass()
    LAST_RESULTS = run_bass_kernel_spmd(
        _BUILT, in_maps, core_ids=list(range(N_CORES))
    )
    full = np.concatenate(
        [LAST_RESULTS.results[c]["o"].reshape(-1) for c in range(N_CORES)]
    )
    return full[oidx].astype(np.float32)
